# revision 11
# baseline (speedup 1.0000x reference)
"""Trainium2 Bass kernel for nn_PitchLoss.

Math: loss = sum(mask * BCEWithLogits(preds, tgt)) / sum(mask), where tgt is a
Gaussian-blurred (5-tap, sigma=0.5, reflect-padded) one-hot of the quantized
pitch bin q = clip(floor((gt-50)/6), 0, 49), and frames with gt == 100 are
masked out (tgt = -1 -> mask 0 on all 50 bins).

Key decomposition: there are only 50 possible target rows, precomputed on host
as Tmat[q, :].  With OH the (masked) one-hot matrix of q over frames:
    S2 = sum_nonpad <preds_row, Tmat[q]>  = <OH^T @ P, Tmat>
    S1 = sum_nonpad softplus(preds)       = sum(OH^T @ softplus(P))
    loss = (S1 - S2) / (50 * n_nonpad)
Both contractions run on the tensor engine accumulating into one PSUM tile;
softplus = Ln(1 + Exp(p)) on the scalar engine (one table set, two passes);
the one-hot comes from one int32 iota-vs-q is_equal per tile on DVE.

Sharding: pure data parallel over the batch axis, 16 batch rows per core.
Per-core output is a tiny [128, 3] partial-sum tensor (sum(A*T) per q-row,
sum(B) per q-row, pad count per partition); final reduction on host.
"""

import numpy as np
from contextlib import ExitStack

import concourse.bacc as bacc
import concourse.tile as tile
from concourse import mybir
from concourse.bass_utils import run_bass_kernel_spmd

# ---- problem constants (hardcoded from spec) ----
B, T, NB = 128, 4096, 50
N_CORES = 8
B_PER_CORE = B // N_CORES          # 16
FRAMES = B_PER_CORE * T            # 65536 frames per core
P = 128                            # SBUF partitions
S = FRAMES // P                    # 512 frames per partition
F = 64                             # frames per partition per tile
NTILES = S // F                    # 8
PAD_VAL = 100                      # gt value marking unvoiced/pad frames
KS, SIGMA = 5, 0.5

f32 = mybir.dt.float32
bf16 = mybir.dt.bfloat16
i32 = mybir.dt.int32
Alu = mybir.AluOpType
Act = mybir.ActivationFunctionType


def _gauss_table() -> np.ndarray:
    """Tmat[q, j] = blurred one-hot target row for quantized bin q (50x50)."""
    x = np.arange(KS, dtype=np.float64) - (KS - 1) / 2.0
    k = np.exp(-0.5 * (x / SIGMA) ** 2)
    k = (k / k.sum()).astype(np.float32)
    tm = np.zeros((NB, NB), dtype=np.float32)
    for q in range(NB):
        oh = np.zeros(NB, np.float32)
        oh[q] = 1.0
        ohp = np.pad(oh, (KS // 2, KS // 2), mode="reflect")
        for j in range(NB):
            tm[q, j] = float(np.dot(k, ohp[j:j + KS]))
    return tm


def _patch_act_tables():
    """Make the ACT table-load inserter pick the combined exp+ln set for both
    Exp and Ln (otherwise it alternates between two sets, reloading the
    ~1.3us table before almost every activation).  We only edit the
    *selection* membership list passed to the inserter; the emitted
    act_func_set_id still indexes the real act_info.json, and the combined
    set genuinely contains both functions."""
    import concourse.bacc as _bacc
    from concourse.hw_specs import get_activation_tables as _orig

    if getattr(_bacc, "_pitchloss_act_patch", False):
        return

    def patched(module_arch):
        tabs = _orig(module_arch)
        both = {name for name, funcs in tabs.items()
                if Act.Exp in funcs and Act.Ln in funcs}
        if both:
            for name, funcs in tabs.items():
                if name not in both:
                    funcs.discard(Act.Exp)
                    funcs.discard(Act.Ln)
        return tabs

    _bacc.get_activation_tables = patched
    _bacc._pitchloss_act_patch = True


WORK_BUFS = 4
N_GPSIMD_EQ = 0   # how many tiles' is_equal runs on gpsimd instead of DVE


def build_program(krep: int = 1):
    """Build the per-core Bass program (SPMD: same NEFF on all 8 cores).

    krep > 1 wraps the body in a For_i loop (for timing measurements only).
    """
    _patch_act_tables()
    nc = bacc.Bacc(
        "TRN2",
        target_bir_lowering=False,
        debug=False,
        enable_asserts=False,
        num_devices=N_CORES,
    )
    preds_h = nc.dram_tensor("preds", [FRAMES, NB], f32, kind="ExternalInput")
    gt_h = nc.dram_tensor("gt", [P, S], i32, kind="ExternalInput")
    tmat_h = nc.dram_tensor("tmat", [NB, NB], f32, kind="ExternalInput")
    out_h = nc.dram_tensor("out", [P, 3], f32, kind="ExternalOutput")

    # frame(p, s) = p*S + s ; tile t covers s in [t*F, (t+1)*F)
    preds_r = preds_h.ap().rearrange("(p s) j -> p s j", p=P)

    with tile.TileContext(nc) as tc, ExitStack() as ctx:
        const = ctx.enter_context(tc.tile_pool(name="const", bufs=1))
        work = ctx.enter_context(tc.tile_pool(name="work", bufs=WORK_BUFS))
        psum = ctx.enter_context(tc.tile_pool(name="psum", bufs=1, space="PSUM"))
        small = ctx.enter_context(tc.tile_pool(name="small", bufs=2))

        # j-index pattern, periodic with period NB along the free dim
        jt = const.tile([P, F, NB], i32)
        nc.gpsimd.iota(jt[:, :, :], pattern=[[0, F], [1, NB]], base=0,
                       channel_multiplier=0)
        tmat_sb = const.tile([NB, NB], f32)
        nc.sync.dma_start(out=tmat_sb, in_=tmat_h.ap())

        def body():
            # ---- quantize gt -> q (exact integer arithmetic) ----
            gt_sb = small.tile([P, S], i32)
            nc.sync.dma_start(out=gt_sb, in_=gt_h.ap())
            h = small.tile([P, S], i32)
            # h = max(gt - 50, 0)
            nc.vector.tensor_scalar(out=h, in0=gt_sb, scalar1=-50, scalar2=0,
                                    op0=Alu.add, op1=Alu.max)
            hm = small.tile([P, S], i32)
            # exact floor(h/6) for h in [0, 149]: (h*10923) >> 16
            nc.vector.tensor_scalar(out=hm, in0=h, scalar1=10923, scalar2=None,
                                    op0=Alu.mult)
            q0a = small.tile([P, S], i32)
            nc.vector.tensor_scalar(out=q0a, in0=hm, scalar1=16, scalar2=None,
                                    op0=Alu.arith_shift_right)
            q0 = small.tile([P, S], i32)
            nc.vector.tensor_scalar(out=q0, in0=q0a, scalar1=49, scalar2=None,
                                    op0=Alu.min)
            padm = small.tile([P, S], i32)
            nc.vector.tensor_scalar(out=padm, in0=gt_sb, scalar1=PAD_VAL,
                                    scalar2=None, op0=Alu.is_equal)
            qf = small.tile([P, S], i32)
            # padded frames get q += 1000 -> never matches the iota -> zero row
            nc.vector.scalar_tensor_tensor(out=qf, in0=padm, scalar=1000,
                                           in1=q0, op0=Alu.mult, op1=Alu.add)
            padred = small.tile([P, 1], f32)
            nc.vector.tensor_reduce(out=padred, in_=padm,
                                    axis=mybir.AxisListType.X, op=Alu.add)

            # ---- main streaming loop ----
            ps_ab = psum.tile([NB, 2, NB], f32)   # [q, {A,B}, j] accumulator
            for t in range(NTILES):
                ps = work.tile([P, 2, F, NB], bf16, name="ps")
                # preds arrive bf16 via SWDGE cast-during-DMA (f32 in HBM)
                nc.gpsimd.dma_start(out=ps[:, 0],
                                    in_=preds_r[:, t * F:(t + 1) * F, :])
                # softplus(p) = Ln(1 + Exp(p)); Exp+Ln share one ACT table set
                et = work.tile([P, F, NB], f32, name="et")
                nc.scalar.activation(out=et, in_=ps[:, 0], func=Act.Exp)
                nc.scalar.activation(out=ps[:, 1], in_=et, func=Act.Ln, bias=1.0)
                oh = work.tile([P, F, NB], bf16, name="oh")
                q_bc = qf[:, t * F:(t + 1) * F][:, :, None].broadcast_to([P, F, NB])
                eq_eng = nc.gpsimd if t < N_GPSIMD_EQ else nc.vector
                eq_eng.tensor_tensor(out=oh, in0=jt, in1=q_bc, op=Alu.is_equal)
                for f in range(F):
                    nc.tensor.matmul(
                        ps_ab, oh[:, f, :], ps[:, :, f, :],
                        start=(t == 0 and f == 0),
                        stop=(t == NTILES - 1 and f == F - 1),
                    )

            # ---- tail reduction ----
            ab = small.tile([NB, 2, NB], f32)
            nc.vector.tensor_copy(out=ab, in_=ps_ab)
            tmp = small.tile([NB, NB], f32)
            nc.vector.tensor_tensor(out=tmp, in0=ab[:, 0], in1=tmat_sb,
                                    op=Alu.mult)
            red = small.tile([P, 3], f32)
            nc.vector.memset(red, 0.0)
            nc.vector.tensor_reduce(out=red[0:NB, 0:1], in_=tmp,
                                    axis=mybir.AxisListType.X, op=Alu.add)
            nc.vector.tensor_reduce(out=red[0:NB, 1:2], in_=ab[:, 1],
                                    axis=mybir.AxisListType.X, op=Alu.add)
            nc.vector.tensor_copy(out=red[:, 2:3], in_=padred)
            nc.sync.dma_start(out=out_h.ap(), in_=red)

        if krep == 1:
            body()
        else:
            with tc.For_i(0, krep, 1):
                body()

    nc.compile()
    return nc


_PROGRAMS: dict[int, object] = {}
_TMAT = None


def _get_program(krep: int = 1):
    if krep not in _PROGRAMS:
        _PROGRAMS[krep] = build_program(krep)
    return _PROGRAMS[krep]


def make_in_maps(preds: np.ndarray, gt: np.ndarray) -> list[dict]:
    global _TMAT
    if _TMAT is None:
        _TMAT = _gauss_table()
    in_maps = []
    for c in range(N_CORES):
        sl = slice(c * B_PER_CORE, (c + 1) * B_PER_CORE)
        in_maps.append({
            "preds": np.ascontiguousarray(preds[sl].reshape(FRAMES, NB)),
            "gt": np.ascontiguousarray(gt[sl].reshape(P, S)),
            "tmat": _TMAT,
        })
    return in_maps


def reduce_outs(outs: list[np.ndarray]) -> np.ndarray:
    s2 = s1 = pad = 0.0
    for o in outs:
        o = o.astype(np.float64)
        s2 += o[:, 0].sum()
        s1 += o[:, 1].sum()
        pad += o[:, 2].sum()
    count = (float(B * T) - pad) * NB
    return np.array((s1 - s2) / count, dtype=np.float32)


def kernel(preds: np.ndarray, gt: np.ndarray) -> np.ndarray:
    preds = np.asarray(preds, dtype=np.float32)
    gt = np.asarray(gt, dtype=np.int32)
    nc = _get_program(1)
    in_maps = make_in_maps(preds, gt)
    res = run_bass_kernel_spmd(nc, in_maps, core_ids=list(range(N_CORES)))
    return reduce_outs([res.results[c]["out"] for c in range(N_CORES)])
